# revision 35
# baseline (speedup 1.0000x reference)
"""Multi-head attention (B=2, SQ=SK=2048, D=1024, H=16, DK=64) on 8 TRN2 cores.

Sharding: core c handles batch b = c//4 and head-group hg = c%4 (4 heads,
256 feature columns of each projection).  Each core computes its heads'
Q/K/V projections, causal+padding-masked softmax attention, and a partial
output projection; the host sums the 4 partials per batch.

All matmul operands are bf16 (1 cycle/row on the PE).  Device layouts:
  qT/kT  [128, m, tok]  packed: feature block m holds heads 2m (partitions
                        0-63) and 2m+1 (64-127) -- exactly the projection
                        psum layout, so evictions are plain copies.
  v      [tok, dk+1]    natural per head, padding mask folded into the rows;
                        the extra "masked ones" column makes the AV matmul
                        emit the softmax denominator for free.
  sT     [ktok, qtok]   transposed scores in PSUM; the two heads of a pair
                        run as K=64 row-tiled matmuls (partitions 0-63 /
                        64-127) that execute concurrently in the PE array.
  ctxT   [65, qtok]     accumulated over ktok tiles (row 64 = denominator).

Causality is exploited at 128-token granularity: score/AV/exp work for a
k-tile only covers valid queries (free dim trimmed), and the diagonal
128x128 triangle is zeroed via affine_select after exp.  Softmax runs
without max subtraction (scores are O(6) for randn inputs).  The Q
projection is emitted per 512-token chunk, interleaved with attention, so
the scalar engine's exp stream starts early; the output projection of
chunk qc-1 is interleaved into chunk qc's attention to fill PE gaps.
"""

import numpy as np

B, SQ, SK, D, H, DK = 2, 2048, 2048, 1024, 16, 64
N_CORES = 8
CORES_PER_BATCH = 4
DKC = D // CORES_PER_BATCH          # 256 projection columns per core
QCH = 512                           # q-chunk (moving free dim)
ONES_EPS = 1e-20

_PROG_CACHE = {}


def _build(cfg):
    """Build the per-core Bass program. cfg = (sq, sk, d, dkc)."""
    import concourse.bass as bass  # noqa: F401
    import concourse.mybir as mybir
    import concourse.tile as tile
    from concourse import bacc
    from contextlib import ExitStack

    f32 = mybir.dt.float32
    bf16 = mybir.dt.bfloat16
    i32 = mybir.dt.int32
    Exp = mybir.ActivationFunctionType.Exp
    mult = mybir.AluOpType.mult
    is_ge = mybir.AluOpType.is_ge

    sq, sk, d, dkc = cfg
    kc_n = d // 128                  # contraction chunks for projections
    mc_n = dkc // 128                # head pairs (128-feature blocks)
    kt_n = sk // 128                 # key tiles
    qc_n = sq // QCH                 # q chunks
    hpc = dkc // DK                  # heads per core
    vw = DK + 1                      # v row width per head incl. ones col
    fc_n = d // 512                  # output feature chunks

    nc = bacc.Bacc("TRN2", target_bir_lowering=False, debug=False,
                   enable_asserts=False, num_devices=N_CORES)

    xqT = nc.dram_tensor("xqT", [d, sq], bf16, kind="ExternalInput").ap()
    xkT = nc.dram_tensor("xkT", [d, sk], bf16, kind="ExternalInput").ap()
    xvT = nc.dram_tensor("xvT", [d, sk], bf16, kind="ExternalInput").ap()
    wq_d = nc.dram_tensor("wq", [d, dkc], bf16, kind="ExternalInput").ap()
    wk_d = nc.dram_tensor("wk", [d, dkc], bf16, kind="ExternalInput").ap()
    wv_d = nc.dram_tensor("wv", [d, dkc], bf16, kind="ExternalInput").ap()
    wo_d = nc.dram_tensor("wo", [dkc, d], bf16, kind="ExternalInput").ap()
    mask_d = nc.dram_tensor("maskb", [sk], i32, kind="ExternalInput").ap()
    out_d = nc.dram_tensor("out", [sq, d], bf16, kind="ExternalOutput").ap()

    with tile.TileContext(nc) as tc, ExitStack() as ctx:
        const = ctx.enter_context(tc.tile_pool(name="const", bufs=1))
        wpool = ctx.enter_context(tc.tile_pool(name="wpool", bufs=4))
        xpool = ctx.enter_context(tc.tile_pool(name="xpool", bufs=1))
        ptp = ctx.enter_context(tc.tile_pool(name="ptp", bufs=4))
        outp = ctx.enter_context(tc.tile_pool(name="outp", bufs=2))
        nrm = ctx.enter_context(tc.tile_pool(name="nrm", bufs=2))
        cbp = ctx.enter_context(tc.tile_pool(name="cbp", bufs=2))
        sbp = ctx.enter_context(tc.tile_pool(name="sbp", bufs=3,
                                             space="PSUM"))
        ctp = ctx.enter_context(tc.tile_pool(name="ctp", bufs=2,
                                             space="PSUM"))

        # ---------------- DMA everything up-front.  xv is issued from the
        # (otherwise idle) vector engine at fine grain so the V projection
        # starts early; xk/xq go on sync after the weights.  Each dma_start
        # lands on one HW queue (~22 GB/s), so tensors are split into many
        # sub-DMAs that spread across the 16 queues.
        mask_i = const.tile([128, kt_n], i32, tag="mask_i")
        nc.sync.dma_start(mask_i[:], mask_d.rearrange("(t p) -> p t", p=128))
        wv_sb = wpool.tile([128, kc_n, dkc], bf16, tag="w")
        nc.sync.dma_start(wv_sb[:], wv_d.rearrange("(c p) m -> p c m", p=128))
        wk_sb = wpool.tile([128, kc_n, dkc], bf16, tag="w")
        nc.sync.dma_start(wk_sb[:], wk_d.rearrange("(c p) m -> p c m", p=128))
        wq_sb = wpool.tile([128, kc_n, dkc], bf16, tag="w")
        nc.sync.dma_start(wq_sb[:], wq_d.rearrange("(c p) m -> p c m", p=128))
        wo_sb = wpool.tile([128, mc_n, fc_n, 512], bf16, tag="w")
        nc.sync.dma_start(wo_sb[:], wo_d.rearrange("(c p) (f n) -> p c f n",
                                                   p=128, n=512))

        _xt = [0]

        def alloc_x(ntok, tok_sub):
            """Allocate per-(c, tok_sub-token) tiles; return (entries at
            512-token granularity, deferred dma-issue callbacks)."""
            nt = ntok // tok_sub
            sub = tok_sub // 512
            xs = [[None] * (ntok // 512) for _ in range(kc_n)]
            tiles = []
            for t in range(nt):
                for c in range(kc_n):
                    _xt[0] += 1
                    tl = xpool.tile([128, tok_sub], bf16, tag=f"x{_xt[0]}",
                                    name="xc")
                    tiles.append((tl, c, t))
                    for s in range(sub):
                        xs[c][t * sub + s] = (tl, s * 512)
            return xs, tiles

        def issue_x(eng, x_dram, tok_sub, tl, c, t):
            eng.dma_start(tl[:], x_dram[c * 128:(c + 1) * 128,
                                        t * tok_sub:(t + 1) * tok_sub])

        # xv: fine-grained, first half issued on sync, second half plus all
        # of xk/xq on the scalar queue (idle early; exp only starts once
        # attention begins) so issue serialization doesn't gate arrival
        xv, xv_tiles = alloc_x(sk, 512)
        for i, (tl, c, t) in enumerate(xv_tiles):
            issue_x(nc.sync if i % 2 == 0 else nc.scalar, xvT, 512, tl, c, t)
        ksub = min(1024, sk)
        qsub = min(1024, sq)
        xk, xk_tiles = alloc_x(sk, ksub)
        xq, xq_tiles = alloc_x(sq, qsub)
        for a in xk_tiles:
            issue_x(nc.scalar, xkT, ksub, *a)
        for a in xq_tiles:
            issue_x(nc.scalar, xqT, qsub, *a)

        # ---------------- constants / persistent tensors
        mask01 = const.tile([128, kt_n], f32, tag="mask01")
        nc.vector.tensor_copy(mask01[:], mask_i[:])
        mask01p = const.tile([128, kt_n], f32, tag="mask01p")
        nc.vector.tensor_scalar_add(mask01p[:], mask01[:], ONES_EPS)

        kTc = [const.tile([128, mc_n, 512], bf16, tag=f"kT{g}",
                          name=f"kT{g}") for g in range(sk // 512)]
        qTc = [const.tile([128, mc_n, QCH], bf16, tag=f"qT{qc}",
                          name=f"qT{qc}") for qc in range(qc_n)]
        vc = [const.tile([128, 4, hpc, vw], bf16, tag=f"v{g}",
                         name=f"v{g}") for g in range(kt_n // 4)]
        cxc = [const.tile([128, mc_n, QCH], bf16, tag=f"cx{qc}",
                          name=f"cx{qc}") for qc in range(qc_n)]

        # ---------------- V projection unit (one 128-token tile; natural
        # layout, padding mask folded in; everything off the scalar engine)
        def vproj_t(t):
            pv = sbp.tile([128, dkc], f32, tag="s", name="pv")
            for c in range(kc_n):
                xt, c0 = xv[c][t // 4]
                o = c0 + (t % 4) * 128
                nc.tensor.matmul(pv[:], xt[:, o:o + 128],
                                 wv_sb[:, c, :],
                                 start=(c == 0), stop=(c == kc_n - 1))
            nc.vector.tensor_scalar(
                out=vc[t // 4][:, t % 4, :, 0:DK],
                in0=pv[:].rearrange("p (h k) -> p h k", h=hpc),
                scalar1=mask01[:, t:t + 1], scalar2=None, op0=mult)
            nc.gpsimd.tensor_copy(
                vc[t // 4][:, t % 4, :, DK:vw],
                mask01p[:, t:t + 1].unsqueeze(1).broadcast_to([128, hpc, 1]))

        # ---------------- K projection unit (one 512-token chunk, one
        # feature block; packed [feature, tok] layout, plain-copy eviction)
        def kproj_qc(qc):
            for m in range(mc_n):
                pk = sbp.tile([128, 512], f32, tag="s", name="pk")
                for c in range(kc_n):
                    xt, c0 = xk[c][qc]
                    nc.tensor.matmul(
                        pk[:], wk_sb[:, c, m * 128:(m + 1) * 128],
                        xt[:, c0:c0 + 512],
                        start=(c == 0), stop=(c == kc_n - 1))
                nc.vector.tensor_copy(kTc[qc][:, m, :], pk[:])

        # ---------------- Q projection for one 512-chunk
        def qproj_qc(qc):
            for m in range(mc_n):
                pk = sbp.tile([128, 512], f32, tag="s", name="pk")
                for c in range(kc_n):
                    xt, c0 = xq[c][qc]
                    nc.tensor.matmul(
                        pk[:], wq_sb[:, c, m * 128:(m + 1) * 128],
                        xt[:, c0:c0 + 512],
                        start=(c == 0), stop=(c == kc_n - 1))
                nc.vector.tensor_copy(qTc[qc][:, m, :], pk[:])

        # ---------------- attention for one 512-chunk, one head pair.
        # Returns list of emit-callbacks so oproj work can be interleaved.
        def attn_pair(qc, m):
            q0 = qc * QCH
            nkt = (q0 + QCH) // 128
            ctxs = [ctp.tile([vw, QCH], f32, tag="c", name="cx") for _ in (0, 1)]
            deferred = []

            def mk_av(pB, kt, off):
                def go():
                    for hh in (0, 1):
                        nc.tensor.matmul(
                            ctxs[hh][:, off:QCH],
                            vc[kt // 4][:, kt % 4, 2 * m + hh, :],
                            pB[:, hh, off:QCH],
                            start=(kt == 0), stop=(kt == nkt - 1),
                            skip_group_check=True)
                return go

            for kt in range(nkt):
                wp = min(QCH, q0 + QCH - kt * 128)   # valid q width
                off = QCH - wp
                sB = sbp.tile([128, 2, QCH], f32, tag="s", name="sB")
                for hh in (0, 1):
                    nc.tensor.matmul(
                        sB[:, hh, off:QCH],
                        kTc[kt // 4][hh * 64:(hh + 1) * 64, m,
                                     (kt % 4) * 128:(kt % 4 + 1) * 128],
                        qTc[qc][hh * 64:(hh + 1) * 64, m, off:QCH],
                        start=True, stop=True)
                pB = ptp.tile([128, 2, QCH], bf16, tag="p", name="pB")
                nc.scalar.activation(pB[:, :, off:QCH], sB[:, :, off:QCH],
                                     Exp, scale=0.125)
                if kt >= nkt - 4:
                    nc.gpsimd.affine_select(
                        out=pB[:, :, off:off + 128],
                        in_=pB[:, :, off:off + 128],
                        compare_op=is_ge, fill=0.0,
                        base=0, channel_multiplier=-1,
                        pattern=[[0, 2], [1, 128]])
                deferred.append(mk_av(pB, kt, off))
                while len(deferred) > 2:
                    deferred.pop(0)()
            for fn in deferred:
                fn()
            # quick-evict ctx PSUM to SBUF (frees the bank for the next
            # pair), then normalize off the critical path:
            # reciprocal of the denominator row -> gpsimd broadcast -> scale
            for hh in (0, 1):
                cb = cbp.tile([vw, QCH], f32, tag="cb", name="cb")
                nc.vector.tensor_copy(cb[:], ctxs[hh][:])
                dn = nrm.tile([1, QCH], f32, tag="dn", name="dn")
                nc.vector.tensor_copy(dn[:], cb[DK:DK + 1, :])
                rc = nrm.tile([1, QCH], f32, tag="rc", name="rc")
                nc.vector.reciprocal_approx_fast(rc[:], dn[:])
                bc = nrm.tile([64, QCH], f32, tag="bc", name="bc")
                nc.gpsimd.partition_broadcast(bc[:], rc[:])
                nc.gpsimd.tensor_tensor(
                    out=cxc[qc][hh * 64:(hh + 1) * 64, m, :],
                    in0=cb[0:DK, :], in1=bc[:], op=mult)

        # ---------------- output projection for a 128-token group.
        # Per-fc DMAs spread across queues; the final groups split further
        # so the last transfer does not dominate the kernel tail.
        def oproj_qt(qc, qt, fine=False):
            qg = qc * QCH + qt * 128
            o_sb = outp.tile([128, fc_n, 512], bf16, tag="o", name="o_sb")
            for fc in range(fc_n):
                po = sbp.tile([128, 512], f32, tag="s", name="po")
                for m in range(mc_n):
                    nc.tensor.matmul(
                        po[:], cxc[qc][:, m, qt * 128:(qt + 1) * 128],
                        wo_sb[:, m, fc, :],
                        start=(m == 0), stop=(m == mc_n - 1))
                nc.vector.tensor_copy(o_sb[:, fc, :], po[:])
                cols = slice(fc * 512, (fc + 1) * 512)
                if fine:
                    for rh in (0, 1):
                        rows = slice(rh * 64, (rh + 1) * 64)
                        nc.sync.dma_start(
                            out_d[qg + rh * 64:qg + (rh + 1) * 64, cols],
                            o_sb[rows, fc, :])
                else:
                    nc.sync.dma_start(out_d[qg:qg + 128, cols],
                                      o_sb[:, fc, :])

        # ---------------- main schedule, fully chunk-pipelined: per q-chunk
        # emit the chunk's V tiles, K chunk, Q chunk, then the head-pair
        # attentions with the previous chunk's output projection interleaved
        for qc in range(qc_n):
            for t in range(4 * qc, min(4 * qc + 4, kt_n)):
                vproj_t(t)
            kproj_qc(qc)
            qproj_qc(qc)
            for m in range(mc_n):
                attn_pair(qc, m)
                if qc > 0:
                    for qt in range(2):
                        oproj_qt(qc - 1, m * 2 + qt)
            if qc > 0 and mc_n == 1:
                for qt in range(2, 4):
                    oproj_qt(qc - 1, qt)
        for qt in range(QCH // 128):
            oproj_qt(qc_n - 1, qt, fine=(qt >= QCH // 128 - 2))
    nc.compile()
    return nc


def _get_program(cfg):
    if cfg not in _PROG_CACHE:
        _PROG_CACHE[cfg] = _build(cfg)
    return _PROG_CACHE[cfg]


def _shard_inputs(query, key, value, mask, Wq, Wk, Wv, Wo):
    """Build the 8 per-core input maps."""
    import ml_dtypes
    f = ml_dtypes.bfloat16
    in_maps = []
    xt = {}
    for b in range(B):
        xt[b] = (np.ascontiguousarray(query[b].T).astype(f),
                 np.ascontiguousarray(key[b].T).astype(f),
                 np.ascontiguousarray(value[b].T).astype(f),
                 np.ascontiguousarray(mask[b], dtype=np.int32))
    for c in range(N_CORES):
        b, hg = divmod(c, CORES_PER_BATCH)
        rows = slice(hg * DKC, (hg + 1) * DKC)
        xq, xk, xv, mb = xt[b]
        in_maps.append({
            "xqT": xq, "xkT": xk, "xvT": xv, "maskb": mb,
            "wq": np.ascontiguousarray(Wq[rows, :].T).astype(f),
            "wk": np.ascontiguousarray(Wk[rows, :].T).astype(f),
            "wv": np.ascontiguousarray(Wv[rows, :].T).astype(f),
            "wo": np.ascontiguousarray(Wo[:, rows].T).astype(f),
        })
    return in_maps


def kernel(query, key, value, mask, Wq, Wk, Wv, Wo):
    from concourse.bass_utils import run_bass_kernel_spmd

    nc = _get_program((SQ, SK, D, DKC))
    in_maps = _shard_inputs(np.asarray(query), np.asarray(key),
                            np.asarray(value), np.asarray(mask),
                            np.asarray(Wq), np.asarray(Wk),
                            np.asarray(Wv), np.asarray(Wo))
    res = run_bass_kernel_spmd(nc, in_maps, list(range(N_CORES)))
    out = np.zeros((B, SQ, D), dtype=np.float32)
    for c in range(N_CORES):
        out[c // CORES_PER_BATCH] += res.results[c]["out"].astype(np.float32)
    return out


# revision 38
# speedup vs baseline: 1.5917x; 1.5917x over previous
"""Multi-head attention (B=2, SQ=SK=2048, D=1024, H=16, DK=64) on 8 TRN2 cores.

Sharding: core c handles batch b = c//4 and head-group hg = c%4 (4 heads,
256 feature columns of each projection).  Each core computes its heads'
Q/K/V projections, causal+padding-masked softmax attention, and a partial
output projection; the host sums the 4 partials per batch.

All matmul operands are bf16 (1 cycle/row on the PE).  Device layouts:
  qT/kT  [128, m, tok]  packed: feature block m holds heads 2m (partitions
                        0-63) and 2m+1 (64-127) -- exactly the projection
                        psum layout, so evictions are plain copies.
  v      [tok, dk+1]    natural per head, padding mask folded into the rows;
                        the extra "masked ones" column makes the AV matmul
                        emit the softmax denominator for free.
  sT     [ktok, qtok]   transposed scores in PSUM; the two heads of a pair
                        run as K=64 row-tiled matmuls (partitions 0-63 /
                        64-127) that execute concurrently in the PE array.
  ctxT   [65, qtok]     accumulated over ktok tiles (row 64 = denominator).

Causality is exploited at 128-token granularity: score/AV/exp work for a
k-tile only covers valid queries (free dim trimmed), and the diagonal
128x128 triangle is zeroed via affine_select after exp.  Softmax runs
without max subtraction (scores are O(6) for randn inputs).  The Q
projection is emitted per 512-token chunk, interleaved with attention, so
the scalar engine's exp stream starts early; the output projection of
chunk qc-1 is interleaved into chunk qc's attention to fill PE gaps.
"""

import numpy as np

B, SQ, SK, D, H, DK = 2, 2048, 2048, 1024, 16, 64
N_CORES = 8
CORES_PER_BATCH = 4
DKC = D // CORES_PER_BATCH          # 256 projection columns per core
QCH = 512                           # q-chunk (moving free dim)
ONES_EPS = 1e-20

_PROG_CACHE = {}


def _build(cfg):
    """Build the per-core Bass program. cfg = (sq, sk, d, dkc)."""
    import concourse.bass as bass  # noqa: F401
    import concourse.mybir as mybir
    import concourse.tile as tile
    from concourse import bacc
    from contextlib import ExitStack

    f32 = mybir.dt.float32
    bf16 = mybir.dt.bfloat16
    i32 = mybir.dt.int32
    Exp = mybir.ActivationFunctionType.Exp
    mult = mybir.AluOpType.mult
    is_ge = mybir.AluOpType.is_ge

    sq, sk, d, dkc = cfg
    kc_n = d // 128                  # contraction chunks for projections
    mc_n = dkc // 128                # head pairs (128-feature blocks)
    kt_n = sk // 128                 # key tiles
    qc_n = sq // QCH                 # q chunks
    hpc = dkc // DK                  # heads per core
    vw = DK + 1                      # v row width per head incl. ones col
    fc_n = d // 512                  # output feature chunks

    nc = bacc.Bacc("TRN2", target_bir_lowering=False, debug=False,
                   enable_asserts=False, num_devices=N_CORES)

    xqT = nc.dram_tensor("xqT", [d, sq], bf16, kind="ExternalInput").ap()
    xkT = nc.dram_tensor("xkT", [d, sk], bf16, kind="ExternalInput").ap()
    xvT = nc.dram_tensor("xvT", [d, sk], bf16, kind="ExternalInput").ap()
    wq_d = nc.dram_tensor("wq", [d, dkc], bf16, kind="ExternalInput").ap()
    wk_d = nc.dram_tensor("wk", [d, dkc], bf16, kind="ExternalInput").ap()
    wv_d = nc.dram_tensor("wv", [d, dkc], bf16, kind="ExternalInput").ap()
    wo_d = nc.dram_tensor("wo", [dkc, d], bf16, kind="ExternalInput").ap()
    mask_d = nc.dram_tensor("maskb", [sk], i32, kind="ExternalInput").ap()
    out_d = nc.dram_tensor("out", [sq, d], bf16, kind="ExternalOutput").ap()

    with tile.TileContext(nc) as tc, ExitStack() as ctx:
        const = ctx.enter_context(tc.tile_pool(name="const", bufs=1))
        wpool = ctx.enter_context(tc.tile_pool(name="wpool", bufs=4))
        xpool = ctx.enter_context(tc.tile_pool(name="xpool", bufs=1))
        ptp = ctx.enter_context(tc.tile_pool(name="ptp", bufs=4))
        outp = ctx.enter_context(tc.tile_pool(name="outp", bufs=2))
        nrm = ctx.enter_context(tc.tile_pool(name="nrm", bufs=2))
        cbp = ctx.enter_context(tc.tile_pool(name="cbp", bufs=2))
        sbp = ctx.enter_context(tc.tile_pool(name="sbp", bufs=3,
                                             space="PSUM"))
        ctp = ctx.enter_context(tc.tile_pool(name="ctp", bufs=2,
                                             space="PSUM"))

        # ---------------- DMA everything up-front.  xv is issued from the
        # (otherwise idle) vector engine at fine grain so the V projection
        # starts early; xk/xq go on sync after the weights.  Each dma_start
        # lands on one HW queue (~22 GB/s), so tensors are split into many
        # sub-DMAs that spread across the 16 queues.
        mask_i = const.tile([128, kt_n], i32, tag="mask_i")
        nc.sync.dma_start(mask_i[:], mask_d.rearrange("(t p) -> p t", p=128))
        wv_sb = wpool.tile([128, kc_n, dkc], bf16, tag="w")
        nc.sync.dma_start(wv_sb[:], wv_d.rearrange("(c p) m -> p c m", p=128))
        wk_sb = wpool.tile([128, kc_n, dkc], bf16, tag="w")
        nc.sync.dma_start(wk_sb[:], wk_d.rearrange("(c p) m -> p c m", p=128))
        wq_sb = wpool.tile([128, kc_n, dkc], bf16, tag="w")
        nc.sync.dma_start(wq_sb[:], wq_d.rearrange("(c p) m -> p c m", p=128))
        wo_sb = wpool.tile([128, mc_n, fc_n, 512], bf16, tag="w")
        nc.sync.dma_start(wo_sb[:], wo_d.rearrange("(c p) (f n) -> p c f n",
                                                   p=128, n=512))

        _xt = [0]

        def alloc_x(ntok, tok_sub):
            """Allocate per-(c, tok_sub-token) tiles; return (entries at
            512-token granularity, deferred dma-issue callbacks)."""
            nt = ntok // tok_sub
            sub = tok_sub // 512
            xs = [[None] * (ntok // 512) for _ in range(kc_n)]
            tiles = []
            for t in range(nt):
                for c in range(kc_n):
                    _xt[0] += 1
                    tl = xpool.tile([128, tok_sub], bf16, tag=f"x{_xt[0]}",
                                    name="xc")
                    tiles.append((tl, c, t))
                    for s in range(sub):
                        xs[c][t * sub + s] = (tl, s * 512)
            return xs, tiles

        def issue_x(eng, x_dram, tok_sub, tl, c, t):
            eng.dma_start(tl[:], x_dram[c * 128:(c + 1) * 128,
                                        t * tok_sub:(t + 1) * tok_sub])

        # All input DMAs issue on sync, interleaved chunk-major across the
        # three X tensors in first-consumption order, so every phase's data
        # is in flight as early as the ~0.6us/issue serial rate allows.
        xv, xv_tiles = alloc_x(sk, 512)
        ksub = min(1024, sk)
        qsub = min(1024, sq)
        xk, xk_tiles = alloc_x(sk, ksub)
        xq, xq_tiles = alloc_x(sq, qsub)

        def tiles_of(tiles, t):
            return [a for a in tiles if a[2] == t]

        issue_order = []
        issue_order += [(xvT, 512, a) for a in tiles_of(xv_tiles, 0)]
        issue_order += [(xkT, ksub, a) for a in tiles_of(xk_tiles, 0)]
        issue_order += [(xqT, qsub, a) for a in tiles_of(xq_tiles, 0)]
        issue_order += [(xvT, 512, a) for a in tiles_of(xv_tiles, 1)]
        issue_order += [(xvT, 512, a) for a in tiles_of(xv_tiles, 2)]
        issue_order += [(xkT, ksub, a) for a in tiles_of(xk_tiles, 1)]
        issue_order += [(xvT, 512, a) for a in tiles_of(xv_tiles, 3)]
        issue_order += [(xqT, qsub, a) for a in tiles_of(xq_tiles, 1)]
        seen = {id(a) for _, _, a in issue_order}
        for tiles, dram, sub in ((xv_tiles, xvT, 512), (xk_tiles, xkT, ksub),
                                 (xq_tiles, xqT, qsub)):
            issue_order += [(dram, sub, a) for a in tiles
                            if id(a) not in seen]
        for dram, sub, a in issue_order:
            issue_x(nc.sync, dram, sub, *a)

        # ---------------- constants / persistent tensors
        mask01 = const.tile([128, kt_n], f32, tag="mask01")
        nc.vector.tensor_copy(mask01[:], mask_i[:])
        mask01p = const.tile([128, kt_n], f32, tag="mask01p")
        nc.vector.tensor_scalar_add(mask01p[:], mask01[:], ONES_EPS)

        kTc = [const.tile([128, mc_n, 512], bf16, tag=f"kT{g}",
                          name=f"kT{g}") for g in range(sk // 512)]
        qTc = [const.tile([128, mc_n, QCH], bf16, tag=f"qT{qc}",
                          name=f"qT{qc}") for qc in range(qc_n)]
        vc = [const.tile([128, 4, hpc, vw], bf16, tag=f"v{g}",
                         name=f"v{g}") for g in range(kt_n // 4)]
        cxc = [const.tile([128, mc_n, QCH], bf16, tag=f"cx{qc}",
                          name=f"cx{qc}") for qc in range(qc_n)]

        # ---------------- V projection unit (one 128-token tile; natural
        # layout, padding mask folded in; everything off the scalar engine)
        def vproj_t(t):
            pv = sbp.tile([128, dkc], f32, tag="s", name="pv")
            for c in range(kc_n):
                xt, c0 = xv[c][t // 4]
                o = c0 + (t % 4) * 128
                nc.tensor.matmul(pv[:], xt[:, o:o + 128],
                                 wv_sb[:, c, :],
                                 start=(c == 0), stop=(c == kc_n - 1))
            nc.vector.tensor_scalar(
                out=vc[t // 4][:, t % 4, :, 0:DK],
                in0=pv[:].rearrange("p (h k) -> p h k", h=hpc),
                scalar1=mask01[:, t:t + 1], scalar2=None, op0=mult)
            nc.vector.tensor_copy(
                vc[t // 4][:, t % 4, :, DK:vw],
                mask01p[:, t:t + 1].unsqueeze(1).broadcast_to([128, hpc, 1]))

        # ---------------- K projection unit (one 512-token chunk, one
        # feature block; packed [feature, tok] layout, plain-copy eviction)
        def kproj_qc(qc):
            for m in range(mc_n):
                pk = sbp.tile([128, 512], f32, tag="s", name="pk")
                for c in range(kc_n):
                    xt, c0 = xk[c][qc]
                    nc.tensor.matmul(
                        pk[:], wk_sb[:, c, m * 128:(m + 1) * 128],
                        xt[:, c0:c0 + 512],
                        start=(c == 0), stop=(c == kc_n - 1))
                nc.vector.tensor_copy(kTc[qc][:, m, :], pk[:])

        # ---------------- Q projection for one 512-chunk
        def qproj_qc(qc):
            for m in range(mc_n):
                pk = sbp.tile([128, 512], f32, tag="s", name="pk")
                for c in range(kc_n):
                    xt, c0 = xq[c][qc]
                    nc.tensor.matmul(
                        pk[:], wq_sb[:, c, m * 128:(m + 1) * 128],
                        xt[:, c0:c0 + 512],
                        start=(c == 0), stop=(c == kc_n - 1))
                nc.vector.tensor_copy(qTc[qc][:, m, :], pk[:])

        # ---------------- attention for one 512-chunk, one head pair.
        # Returns list of emit-callbacks so oproj work can be interleaved.
        def attn_pair(qc, m):
            q0 = qc * QCH
            nkt = (q0 + QCH) // 128
            ctxs = [ctp.tile([vw, QCH], f32, tag="c", name="cx") for _ in (0, 1)]
            deferred = []

            def mk_av(pB, kt, off):
                def go():
                    for hh in (0, 1):
                        nc.tensor.matmul(
                            ctxs[hh][:, off:QCH],
                            vc[kt // 4][:, kt % 4, 2 * m + hh, :],
                            pB[:, hh, off:QCH],
                            start=(kt == 0), stop=(kt == nkt - 1),
                            skip_group_check=True)
                return go

            for kt in range(nkt):
                wp = min(QCH, q0 + QCH - kt * 128)   # valid q width
                off = QCH - wp
                sB = sbp.tile([128, 2, QCH], f32, tag="s", name="sB")
                for hh in (0, 1):
                    nc.tensor.matmul(
                        sB[:, hh, off:QCH],
                        kTc[kt // 4][hh * 64:(hh + 1) * 64, m,
                                     (kt % 4) * 128:(kt % 4 + 1) * 128],
                        qTc[qc][hh * 64:(hh + 1) * 64, m, off:QCH],
                        start=True, stop=True)
                pB = ptp.tile([128, 2, QCH], bf16, tag="p", name="pB")
                nc.scalar.activation(pB[:, :, off:QCH], sB[:, :, off:QCH],
                                     Exp, scale=0.125)
                if kt >= nkt - 4:
                    nc.gpsimd.affine_select(
                        out=pB[:, :, off:off + 128],
                        in_=pB[:, :, off:off + 128],
                        compare_op=is_ge, fill=0.0,
                        base=0, channel_multiplier=-1,
                        pattern=[[0, 2], [1, 128]])
                deferred.append(mk_av(pB, kt, off))
                while len(deferred) > 2:
                    deferred.pop(0)()
            for fn in deferred:
                fn()
            # quick-evict ctx PSUM to SBUF (frees the bank for the next
            # pair), then normalize off the critical path:
            # reciprocal of the denominator row -> gpsimd broadcast -> scale
            for hh in (0, 1):
                cb = cbp.tile([vw, QCH], f32, tag="cb", name="cb")
                nc.vector.tensor_copy(cb[:], ctxs[hh][:])
                dn = nrm.tile([1, QCH], f32, tag="dn", name="dn")
                nc.vector.tensor_copy(dn[:], cb[DK:DK + 1, :])
                rc = nrm.tile([1, QCH], f32, tag="rc", name="rc")
                nc.vector.reciprocal_approx_fast(rc[:], dn[:])
                bc = nrm.tile([64, QCH], f32, tag="bc", name="bc")
                nc.gpsimd.partition_broadcast(bc[:], rc[:])
                nc.vector.tensor_tensor(
                    out=cxc[qc][hh * 64:(hh + 1) * 64, m, :],
                    in0=cb[0:DK, :], in1=bc[:], op=mult)

        # ---------------- output projection for a 128-token group.
        # Per-fc DMAs spread across queues; the final groups split further
        # so the last transfer does not dominate the kernel tail.
        def oproj_qt(qc, qt, fine=False):
            qg = qc * QCH + qt * 128
            o_sb = outp.tile([128, fc_n, 512], bf16, tag="o", name="o_sb")
            for fc in range(fc_n):
                po = sbp.tile([128, 512], f32, tag="s", name="po")
                for m in range(mc_n):
                    nc.tensor.matmul(
                        po[:], cxc[qc][:, m, qt * 128:(qt + 1) * 128],
                        wo_sb[:, m, fc, :],
                        start=(m == 0), stop=(m == mc_n - 1))
                nc.vector.tensor_copy(o_sb[:, fc, :], po[:])
                cols = slice(fc * 512, (fc + 1) * 512)
                if fine:
                    for rh in (0, 1):
                        rows = slice(rh * 64, (rh + 1) * 64)
                        nc.sync.dma_start(
                            out_d[qg + rh * 64:qg + (rh + 1) * 64, cols],
                            o_sb[rows, fc, :])
                else:
                    nc.sync.dma_start(out_d[qg:qg + 128, cols],
                                      o_sb[:, fc, :])

        # ---------------- main schedule, fully chunk-pipelined: per q-chunk
        # emit the chunk's V tiles, K chunk, Q chunk, then the head-pair
        # attentions with the previous chunk's output projection interleaved
        for qc in range(qc_n):
            for t in range(4 * qc, min(4 * qc + 4, kt_n)):
                vproj_t(t)
            kproj_qc(qc)
            qproj_qc(qc)
            for m in range(mc_n):
                attn_pair(qc, m)
                if qc > 0:
                    for qt in range(2):
                        oproj_qt(qc - 1, m * 2 + qt)
            if qc > 0 and mc_n == 1:
                for qt in range(2, 4):
                    oproj_qt(qc - 1, qt)
        for qt in range(QCH // 128):
            oproj_qt(qc_n - 1, qt, fine=(qt >= QCH // 128 - 2))
    nc.compile()
    return nc


def _get_program(cfg):
    if cfg not in _PROG_CACHE:
        _PROG_CACHE[cfg] = _build(cfg)
    return _PROG_CACHE[cfg]


def _shard_inputs(query, key, value, mask, Wq, Wk, Wv, Wo):
    """Build the 8 per-core input maps."""
    import ml_dtypes
    f = ml_dtypes.bfloat16
    in_maps = []
    xt = {}
    for b in range(B):
        xt[b] = (np.ascontiguousarray(query[b].T).astype(f),
                 np.ascontiguousarray(key[b].T).astype(f),
                 np.ascontiguousarray(value[b].T).astype(f),
                 np.ascontiguousarray(mask[b], dtype=np.int32))
    for c in range(N_CORES):
        b, hg = divmod(c, CORES_PER_BATCH)
        rows = slice(hg * DKC, (hg + 1) * DKC)
        xq, xk, xv, mb = xt[b]
        in_maps.append({
            "xqT": xq, "xkT": xk, "xvT": xv, "maskb": mb,
            "wq": np.ascontiguousarray(Wq[rows, :].T).astype(f),
            "wk": np.ascontiguousarray(Wk[rows, :].T).astype(f),
            "wv": np.ascontiguousarray(Wv[rows, :].T).astype(f),
            "wo": np.ascontiguousarray(Wo[:, rows].T).astype(f),
        })
    return in_maps


def kernel(query, key, value, mask, Wq, Wk, Wv, Wo):
    from concourse.bass_utils import run_bass_kernel_spmd

    nc = _get_program((SQ, SK, D, DKC))
    in_maps = _shard_inputs(np.asarray(query), np.asarray(key),
                            np.asarray(value), np.asarray(mask),
                            np.asarray(Wq), np.asarray(Wk),
                            np.asarray(Wv), np.asarray(Wo))
    res = run_bass_kernel_spmd(nc, in_maps, list(range(N_CORES)))
    out = np.zeros((B, SQ, D), dtype=np.float32)
    for c in range(N_CORES):
        out[c // CORES_PER_BATCH] += res.results[c]["out"].astype(np.float32)
    return out


# revision 45
# speedup vs baseline: 1.6946x; 1.0646x over previous
"""Multi-head attention (B=2, SQ=SK=2048, D=1024, H=16, DK=64) on 8 TRN2 cores.

Sharding: core c handles batch b = c//4 and head-group hg = c%4 (4 heads,
256 feature columns of each projection).  Each core computes its heads'
Q/K/V projections, causal+padding-masked softmax attention, and a partial
output projection; the host sums the 4 partials per batch.

All matmul operands are bf16 (1 cycle/row on the PE).  Device layouts:
  qT/kT  [128, m, tok]  packed: feature block m holds heads 2m (partitions
                        0-63) and 2m+1 (64-127) -- exactly the projection
                        psum layout, so evictions are plain copies.
  v      [tok, dk+1]    natural per head, padding mask folded into the rows;
                        the extra "masked ones" column makes the AV matmul
                        emit the softmax denominator for free.
  sT     [ktok, qtok]   transposed scores in PSUM; the two heads of a pair
                        run as K=64 row-tiled matmuls (partitions 0-63 /
                        64-127) that execute concurrently in the PE array.
  ctxT   [65, qtok]     accumulated over ktok tiles (row 64 = denominator).

Causality is exploited at 128-token granularity: score/AV/exp work for a
k-tile only covers valid queries (free dim trimmed), and the diagonal
128x128 triangle is zeroed via affine_select after exp.  Softmax runs
without max subtraction (scores are O(6) for randn inputs).  The Q
projection is emitted per 512-token chunk, interleaved with attention, so
the scalar engine's exp stream starts early; the output projection of
chunk qc-1 is interleaved into chunk qc's attention to fill PE gaps.
"""

import numpy as np

B, SQ, SK, D, H, DK = 2, 2048, 2048, 1024, 16, 64
N_CORES = 8
CORES_PER_BATCH = 4
DKC = D // CORES_PER_BATCH          # 256 projection columns per core
QCH = 512                           # q-chunk (moving free dim)
ONES_EPS = 1e-20

_PROG_CACHE = {}


def _build(cfg):
    """Build the per-core Bass program. cfg = (sq, sk, d, dkc)."""
    import concourse.bass as bass  # noqa: F401
    import concourse.mybir as mybir
    import concourse.tile as tile
    from concourse import bacc
    from contextlib import ExitStack

    f32 = mybir.dt.float32
    bf16 = mybir.dt.bfloat16
    i32 = mybir.dt.int32
    Exp = mybir.ActivationFunctionType.Exp
    mult = mybir.AluOpType.mult
    is_ge = mybir.AluOpType.is_ge

    sq, sk, d, dkc = cfg
    kc_n = d // 128                  # contraction chunks for projections
    mc_n = dkc // 128                # head pairs (128-feature blocks)
    kt_n = sk // 128                 # key tiles
    qc_n = sq // QCH                 # q chunks
    hpc = dkc // DK                  # heads per core
    vw = DK + 1                      # v row width per head incl. ones col
    fc_n = d // 512                  # output feature chunks

    nc = bacc.Bacc("TRN2", target_bir_lowering=False, debug=False,
                   enable_asserts=False, num_devices=N_CORES)

    xqT = nc.dram_tensor("xqT", [d, sq], bf16, kind="ExternalInput").ap()
    xkT = nc.dram_tensor("xkT", [d, sk], bf16, kind="ExternalInput").ap()
    xvT = nc.dram_tensor("xvT", [d, sk], bf16, kind="ExternalInput").ap()
    wq_d = nc.dram_tensor("wq", [d, dkc], bf16, kind="ExternalInput").ap()
    wk_d = nc.dram_tensor("wk", [d, dkc], bf16, kind="ExternalInput").ap()
    wv_d = nc.dram_tensor("wv", [d, dkc], bf16, kind="ExternalInput").ap()
    wo_d = nc.dram_tensor("wo", [dkc, d], bf16, kind="ExternalInput").ap()
    mask_d = nc.dram_tensor("maskb", [sk], i32, kind="ExternalInput").ap()
    out_d = nc.dram_tensor("out", [sq, d], bf16, kind="ExternalOutput").ap()

    with tile.TileContext(nc) as tc, ExitStack() as ctx:
        const = ctx.enter_context(tc.tile_pool(name="const", bufs=1))
        wpool = ctx.enter_context(tc.tile_pool(name="wpool", bufs=4))
        xpool = ctx.enter_context(tc.tile_pool(name="xpool", bufs=1))
        ptp = ctx.enter_context(tc.tile_pool(name="ptp", bufs=4))
        outp = ctx.enter_context(tc.tile_pool(name="outp", bufs=2))
        nrm = ctx.enter_context(tc.tile_pool(name="nrm", bufs=2))
        cbp = ctx.enter_context(tc.tile_pool(name="cbp", bufs=2))
        sbp = ctx.enter_context(tc.tile_pool(name="sbp", bufs=2,
                                             space="PSUM"))
        prjp = ctx.enter_context(tc.tile_pool(name="prjp", bufs=2,
                                              space="PSUM"))
        ctp = ctx.enter_context(tc.tile_pool(name="ctp", bufs=2,
                                             space="PSUM"))

        # ---------------- DMA everything up-front.  xv is issued from the
        # (otherwise idle) vector engine at fine grain so the V projection
        # starts early; xk/xq go on sync after the weights.  Each dma_start
        # lands on one HW queue (~22 GB/s), so tensors are split into many
        # sub-DMAs that spread across the 16 queues.
        mask_i = const.tile([128, kt_n], i32, tag="mask_i")
        nc.sync.dma_start(mask_i[:], mask_d.rearrange("(t p) -> p t", p=128))
        wv_sb = wpool.tile([128, kc_n, dkc], bf16, tag="w")
        nc.sync.dma_start(wv_sb[:], wv_d.rearrange("(c p) m -> p c m", p=128))
        wk_sb = wpool.tile([128, kc_n, dkc], bf16, tag="w")
        nc.sync.dma_start(wk_sb[:], wk_d.rearrange("(c p) m -> p c m", p=128))
        wq_sb = wpool.tile([128, kc_n, dkc], bf16, tag="w")
        nc.sync.dma_start(wq_sb[:], wq_d.rearrange("(c p) m -> p c m", p=128))
        wo_sb = wpool.tile([128, mc_n, fc_n, 512], bf16, tag="w")
        nc.sync.dma_start(wo_sb[:], wo_d.rearrange("(c p) (f n) -> p c f n",
                                                   p=128, n=512))

        _xt = [0]

        def alloc_x(ntok, tok_sub):
            """Allocate per-(c, tok_sub-token) tiles; return (entries at
            512-token granularity, deferred dma-issue callbacks)."""
            nt = ntok // tok_sub
            sub = tok_sub // 512
            xs = [[None] * (ntok // 512) for _ in range(kc_n)]
            tiles = []
            for t in range(nt):
                for c in range(kc_n):
                    _xt[0] += 1
                    tl = xpool.tile([128, tok_sub], bf16, tag=f"x{_xt[0]}",
                                    name="xc")
                    tiles.append((tl, c, t))
                    for s in range(sub):
                        xs[c][t * sub + s] = (tl, s * 512)
            return xs, tiles

        def issue_x(eng, x_dram, tok_sub, tl, c, t):
            eng.dma_start(tl[:], x_dram[c * 128:(c + 1) * 128,
                                        t * tok_sub:(t + 1) * tok_sub])

        # All input DMAs issue on sync, interleaved chunk-major across the
        # three X tensors in first-consumption order, so every phase's data
        # is in flight as early as the ~0.6us/issue serial rate allows.
        xv, xv_tiles = alloc_x(sk, 512)
        ksub = min(1024, sk)
        qsub = min(1024, sq)
        xk, xk_tiles = alloc_x(sk, ksub)
        xq, xq_tiles = alloc_x(sq, qsub)

        def tiles_of(tiles, t):
            return [a for a in tiles if a[2] == t]

        issue_order = []
        issue_order += [(xvT, 512, a) for a in tiles_of(xv_tiles, 0)]
        issue_order += [(xkT, ksub, a) for a in tiles_of(xk_tiles, 0)]
        issue_order += [(xqT, qsub, a) for a in tiles_of(xq_tiles, 0)]
        issue_order += [(xvT, 512, a) for a in tiles_of(xv_tiles, 1)]
        issue_order += [(xvT, 512, a) for a in tiles_of(xv_tiles, 2)]
        issue_order += [(xkT, ksub, a) for a in tiles_of(xk_tiles, 1)]
        issue_order += [(xvT, 512, a) for a in tiles_of(xv_tiles, 3)]
        issue_order += [(xqT, qsub, a) for a in tiles_of(xq_tiles, 1)]
        seen = {id(a) for _, _, a in issue_order}
        for tiles, dram, sub in ((xv_tiles, xvT, 512), (xk_tiles, xkT, ksub),
                                 (xq_tiles, xqT, qsub)):
            issue_order += [(dram, sub, a) for a in tiles
                            if id(a) not in seen]
        for dram, sub, a in issue_order:
            issue_x(nc.sync, dram, sub, *a)

        # ---------------- constants / persistent tensors
        mask01 = const.tile([128, kt_n], f32, tag="mask01")
        nc.vector.tensor_copy(mask01[:], mask_i[:])
        mask01p = const.tile([128, kt_n], f32, tag="mask01p")
        nc.vector.tensor_scalar_add(mask01p[:], mask01[:], ONES_EPS)

        kTc = [const.tile([128, mc_n, 512], bf16, tag=f"kT{g}",
                          name=f"kT{g}") for g in range(sk // 512)]
        qTc = [const.tile([128, mc_n, QCH], bf16, tag=f"qT{qc}",
                          name=f"qT{qc}") for qc in range(qc_n)]
        vc = [const.tile([128, 4, hpc, vw], bf16, tag=f"v{g}",
                         name=f"v{g}") for g in range(kt_n // 4)]
        cxc = [const.tile([128, mc_n, QCH], bf16, tag=f"cx{qc}",
                          name=f"cx{qc}") for qc in range(qc_n)]

        # ---------------- V projection unit (one 128-token tile; natural
        # layout, padding mask folded in; everything off the scalar engine)
        def vproj_t(t):
            pv = prjp.tile([128, dkc], f32, tag="pj", name="pv")
            for c in range(kc_n):
                xt, c0 = xv[c][t // 4]
                o = c0 + (t % 4) * 128
                nc.tensor.matmul(pv[:], xt[:, o:o + 128],
                                 wv_sb[:, c, :],
                                 start=(c == 0), stop=(c == kc_n - 1))
            nc.vector.tensor_scalar(
                out=vc[t // 4][:, t % 4, :, 0:DK],
                in0=pv[:].rearrange("p (h k) -> p h k", h=hpc),
                scalar1=mask01[:, t:t + 1], scalar2=None, op0=mult)
            nc.vector.tensor_copy(
                vc[t // 4][:, t % 4, :, DK:vw],
                mask01p[:, t:t + 1].unsqueeze(1).broadcast_to([128, hpc, 1]))

        # ---------------- K projection unit (one 512-token chunk, one
        # feature block; packed [feature, tok] layout, plain-copy eviction)
        def kproj_u(qc, m):
            pk = prjp.tile([128, 512], f32, tag="pj", name="pk")
            for c in range(kc_n):
                xt, c0 = xk[c][qc]
                nc.tensor.matmul(
                    pk[:], wk_sb[:, c, m * 128:(m + 1) * 128],
                    xt[:, c0:c0 + 512],
                    start=(c == 0), stop=(c == kc_n - 1))
            nc.vector.tensor_copy(kTc[qc][:, m, :], pk[:])

        # ---------------- Q projection for one 512-chunk, one block
        def qproj_u(qc, m):
            pk = prjp.tile([128, 512], f32, tag="pj", name="pk")
            for c in range(kc_n):
                xt, c0 = xq[c][qc]
                nc.tensor.matmul(
                    pk[:], wq_sb[:, c, m * 128:(m + 1) * 128],
                    xt[:, c0:c0 + 512],
                    start=(c == 0), stop=(c == kc_n - 1))
            nc.vector.tensor_copy(qTc[qc][:, m, :], pk[:])

        # ---------------- attention for one 512-chunk, one head pair.
        # `fillers` holds projection/output-projection unit callbacks that
        # are drained between kt units to fill the exp-paced PE slack.
        def attn_pair(qc, m, fillers):
            q0 = qc * QCH
            nkt = (q0 + QCH) // 128
            ctxs = [ctp.tile([vw, QCH], f32, tag="c", name="cx") for _ in (0, 1)]
            deferred = []

            def mk_av(pB, kt, off):
                def go():
                    for hh in (0, 1):
                        nc.tensor.matmul(
                            ctxs[hh][:, off:QCH],
                            vc[kt // 4][:, kt % 4, 2 * m + hh, :],
                            pB[:, hh, off:QCH],
                            start=(kt == 0), stop=(kt == nkt - 1),
                            skip_group_check=True)
                return go

            for kt in range(nkt):
                wp = min(QCH, q0 + QCH - kt * 128)   # valid q width
                off = QCH - wp
                sB = sbp.tile([128, 2, QCH], f32, tag="s", name="sB")
                for hh in (0, 1):
                    nc.tensor.matmul(
                        sB[:, hh, off:QCH],
                        kTc[kt // 4][hh * 64:(hh + 1) * 64, m,
                                     (kt % 4) * 128:(kt % 4 + 1) * 128],
                        qTc[qc][hh * 64:(hh + 1) * 64, m, off:QCH],
                        start=True, stop=True)
                pB = ptp.tile([128, 2, QCH], bf16, tag="p", name="pB")
                nc.scalar.activation(pB[:, :, off:QCH], sB[:, :, off:QCH],
                                     Exp, scale=0.125)
                if kt >= nkt - 4:
                    nc.gpsimd.affine_select(
                        out=pB[:, :, off:off + 128],
                        in_=pB[:, :, off:off + 128],
                        compare_op=is_ge, fill=0.0,
                        base=0, channel_multiplier=-1,
                        pattern=[[0, 2], [1, 128]])
                deferred.append(mk_av(pB, kt, off))
                if fillers:
                    fillers.pop(0)()
                while len(deferred) > 2:
                    deferred.pop(0)()
            for fn in deferred:
                fn()
            # quick-evict ctx PSUM to SBUF (frees the bank for the next
            # pair), then normalize off the critical path:
            # reciprocal of the denominator row -> gpsimd broadcast -> scale
            for hh in (0, 1):
                cb = cbp.tile([vw, QCH], f32, tag="cb", name="cb")
                nc.vector.tensor_copy(cb[:], ctxs[hh][:])
                dn = nrm.tile([1, QCH], f32, tag="dn", name="dn")
                nc.vector.tensor_copy(dn[:], cb[DK:DK + 1, :])
                rc = nrm.tile([1, QCH], f32, tag="rc", name="rc")
                nc.vector.reciprocal_approx_fast(rc[:], dn[:])
                bc = nrm.tile([64, QCH], f32, tag="bc", name="bc")
                nc.gpsimd.partition_broadcast(bc[:], rc[:])
                nc.vector.tensor_tensor(
                    out=cxc[qc][hh * 64:(hh + 1) * 64, m, :],
                    in0=cb[0:DK, :], in1=bc[:], op=mult)

        # ---------------- output projection for a 128-token group.
        # Per-fc DMAs spread across queues; the final groups split further
        # so the last transfer does not dominate the kernel tail.
        def oproj_qt(qc, qt, fine=False):
            qg = qc * QCH + qt * 128
            o_sb = outp.tile([128, fc_n, 512], bf16, tag="o", name="o_sb")
            for fc in range(fc_n):
                po = prjp.tile([128, 512], f32, tag="pj", name="po")
                for m in range(mc_n):
                    nc.tensor.matmul(
                        po[:], cxc[qc][:, m, qt * 128:(qt + 1) * 128],
                        wo_sb[:, m, fc, :],
                        start=(m == 0), stop=(m == mc_n - 1))
                nc.vector.tensor_copy(o_sb[:, fc, :], po[:])
                cols = slice(fc * 512, (fc + 1) * 512)
                if fine:
                    for rh in (0, 1):
                        rows = slice(rh * 64, (rh + 1) * 64)
                        nc.sync.dma_start(
                            out_d[qg + rh * 64:qg + (rh + 1) * 64, cols],
                            o_sb[rows, fc, :])
                else:
                    nc.sync.dma_start(out_d[qg:qg + 128, cols],
                                      o_sb[:, fc, :])

        # ---------------- main schedule: chunk 0's projections up-front;
        # each chunk's attention drains the NEXT chunk's projection units
        # and the PREVIOUS chunk's output projection as fillers between kt
        # units, keeping the PE busy through the exp-paced stretches.
        def proj_units(qc):
            us = [(lambda t=t: vproj_t(t))
                  for t in range(4 * qc, min(4 * qc + 4, kt_n))]
            us += [(lambda m=m: kproj_u(qc, m)) for m in range(mc_n)]
            us += [(lambda m=m: qproj_u(qc, m)) for m in range(mc_n)]
            return us

        for u in proj_units(0):
            u()
        for qc in range(qc_n):
            fillers = []
            if qc > 0:
                fillers += [(lambda qt=qt: oproj_qt(qc - 1, qt))
                            for qt in range(QCH // 128)]
            if qc + 1 < qc_n:
                fillers += proj_units(qc + 1)
            for m in range(mc_n):
                attn_pair(qc, m, fillers)
            for u in fillers:
                u()
        for qt in range(QCH // 128):
            oproj_qt(qc_n - 1, qt, fine=(qt >= QCH // 128 - 2))
    nc.compile()
    return nc


def _get_program(cfg):
    if cfg not in _PROG_CACHE:
        _PROG_CACHE[cfg] = _build(cfg)
    return _PROG_CACHE[cfg]


def _shard_inputs(query, key, value, mask, Wq, Wk, Wv, Wo):
    """Build the 8 per-core input maps."""
    import ml_dtypes
    f = ml_dtypes.bfloat16
    in_maps = []
    xt = {}
    for b in range(B):
        xt[b] = (np.ascontiguousarray(query[b].T).astype(f),
                 np.ascontiguousarray(key[b].T).astype(f),
                 np.ascontiguousarray(value[b].T).astype(f),
                 np.ascontiguousarray(mask[b], dtype=np.int32))
    for c in range(N_CORES):
        b, hg = divmod(c, CORES_PER_BATCH)
        rows = slice(hg * DKC, (hg + 1) * DKC)
        xq, xk, xv, mb = xt[b]
        in_maps.append({
            "xqT": xq, "xkT": xk, "xvT": xv, "maskb": mb,
            "wq": np.ascontiguousarray(Wq[rows, :].T).astype(f),
            "wk": np.ascontiguousarray(Wk[rows, :].T).astype(f),
            "wv": np.ascontiguousarray(Wv[rows, :].T).astype(f),
            "wo": np.ascontiguousarray(Wo[:, rows].T).astype(f),
        })
    return in_maps


def kernel(query, key, value, mask, Wq, Wk, Wv, Wo):
    from concourse.bass_utils import run_bass_kernel_spmd

    nc = _get_program((SQ, SK, D, DKC))
    in_maps = _shard_inputs(np.asarray(query), np.asarray(key),
                            np.asarray(value), np.asarray(mask),
                            np.asarray(Wq), np.asarray(Wk),
                            np.asarray(Wv), np.asarray(Wo))
    res = run_bass_kernel_spmd(nc, in_maps, list(range(N_CORES)))
    out = np.zeros((B, SQ, D), dtype=np.float32)
    for c in range(N_CORES):
        out[c // CORES_PER_BATCH] += res.results[c]["out"].astype(np.float32)
    return out
